# revision 30
# baseline (speedup 1.0000x reference)
"""Trainium2 Bass kernel for nn_Encoder_Decoder_fc (encoder LSTM -> decoder LSTMCell + Linear).

Data-parallel over batch (B=256 -> 32 per core on 8 cores), weights replicated.

Per step, gates live in PSUM as G[32*j + b, c*128 + f] = gate_c[b, 128*j + f]
with gate fold order c = (f, i, o, g).  The step pipeline:

  G = XU-matmul (input+bias, one K=128 matmul)  + h @ Whh^T (fp8 DoubleRow
      matmuls: stationary h^T slab pairs, moving Whh pairs, K=256/pass)
  gate free dim is computed in two halves [f,i | o,g] so sigmoid(f,i) starts
  while the second half of the matmul block is still streaming.
  c = sig(f)*c + sig(i)*tanh(g)   (3 DVE ops, fp32)
  h^T = sig(o)^T * tanh(c)^T built via two PE transposes straight into the
      fp8 ring slab that is the next step's stationary operand.

Modes: "bf16" (default, shipped): 48 bf16 matmuls/step (16 per free block).
"fp8"/"fp8split" use fp8e4m3 DoubleRow matmuls at half cost -- they validate
in CoreSim but are rejected by walrus codegen on this silicon revision
(s3d3_mm_valid_dst_partition: DoubleRow outputs may only target PSUM
partition 0, which is incompatible with the 4-strip col tiling this layout
needs), so they remain sim-only.
"""

import sys

sys.path.insert(0, "/opt/trn_rl_repo")

from contextlib import ExitStack

import numpy as np

import concourse.bass as bass  # noqa: F401  (re-exported for tooling)
import concourse.mybir as mybir
import concourse.tile as tile
from concourse import bacc
from concourse.bass_utils import run_bass_kernel_spmd
from concourse.masks import make_identity

P = 128
H = 512
B = 256
T = 512
N_CORES = 8
BL = B // N_CORES  # 32 batch per core
KC = H // P  # 4 h chunks of 128
JC = 4  # gate-fold partition groups (h slices)
GF = 4 * P  # 512: per-strip gate free size, fold [f | i | o | g]
HGF = GF // 2  # 256: half (f,i) / (o,g)
RING = 32  # h^T ring slabs (2 windows of 16)
WIN = 16  # ys window size (steps)
SPK = 1  # steps per XU stationary block (no packing: see xu_probe)

F32 = mybir.dt.float32
BF16 = mybir.dt.bfloat16
F8 = mybir.dt.float8e4
AF = mybir.ActivationFunctionType
DR = mybir.MatmulPerfMode.DoubleRow
WSCALE = 64.0  # fp8 modes: W,UW stored x64 (avoid e4m3 subnormals), ACT descales

# fold order within free dim: f, i, o, g ; torch row blocks: i,f,g,o
_CBASE = (1 * H, 0 * H, 3 * H, 2 * H)  # f, i, o, g


def _perm_fold() -> np.ndarray:
    """perm[j*GF + c*P + f] = torch row index for folded column (j, c, f)."""
    idx = np.empty(4 * H, dtype=np.int64)
    for j in range(JC):
        for c in range(4):
            base = j * GF + c * P
            idx[base : base + P] = _CBASE[c] + j * P + np.arange(P)
    return idx


def _emit_rec_o(nc, Gs, consts, is_enc, prev_slab, k_lo, k_hi):
    """Deferred o-block rec matmuls (bf16), emitted between chain transposes."""
    sW = consts["WE"] if is_enc else consts["WD"]
    ring = consts["ring_hi"]
    G, lo_c, hi_c = Gs[2], 2 * P, 3 * P
    for k in range(k_lo, k_hi):
        lhsT = ring[:, prev_slab * P + 32 * k : prev_slab * P + 32 * (k + 1)]
        for j in range(JC):
            nc.tensor.matmul(
                G[32 * j : 32 * (j + 1), 0 : hi_c - lo_c],
                lhsT,
                sW[k][:, j * GF + lo_c : j * GF + hi_c],
                start=False,
                stop=(k == KC - 1),
                tile_position=(0, 32 * j),
                skip_group_check=True,
            )


def _emit_mms(nc, Gs, consts, is_enc, t, prev_slab, mode, skip_rec=False, split_o=False):
    """Emit the matmul block for one step.  Gs = (G_fi, G_g, G_o) PSUM tiles.
    Free blocks of the fold: fi = cols 0:256, g = 384:512, o = 256:384.
    All input MMs are emitted first so they run during the previous chain."""
    sXU = consts["XU"]
    sUW = consts["UWE"] if is_enc else consts["UWD"]
    xu = sXU[:, t * P : (t + 1) * P]
    blocks = [(Gs[0], 0, 2 * P), (Gs[1], 3 * P, 4 * P), (Gs[2], 2 * P, 3 * P)]

    for G, lo_c, hi_c in blocks:
        nc.tensor.matmul(
            G[:, 0 : hi_c - lo_c],
            xu,
            sUW[:, lo_c:hi_c],
            start=True,
            stop=skip_rec,
            tile_position=(0, 0),
            skip_group_check=True,
        )
    if skip_rec:
        return
    if mode == "bf16":
        sW = consts["WE"] if is_enc else consts["WD"]
        ring = consts["ring_hi"]
        for G, lo_c, hi_c in blocks[: 2 if split_o else 3]:
            for k in range(KC):
                lhsT = ring[:, prev_slab * P + 32 * k : prev_slab * P + 32 * (k + 1)]
                for j in range(JC):
                    nc.tensor.matmul(
                        G[32 * j : 32 * (j + 1), 0 : hi_c - lo_c],
                        lhsT,
                        sW[k][:, j * GF + lo_c : j * GF + hi_c],
                        start=False,
                        stop=(k == KC - 1),
                        tile_position=(0, 32 * j),
                        skip_group_check=True,
                    )
    else:
        sW8 = consts["W8E"] if is_enc else consts["W8D"]
        rings = [("ring_hi", sW8)]
        if mode == "fp8split":
            sW8L = consts["W8LE"] if is_enc else consts["W8LD"]
            rings.append(("ring_lo", sW8L))
        n_stage = len(rings)
        for G, lo_c, hi_c in blocks:
            for st, (rname, w8) in enumerate(rings):
                ring = consts[rname]
                for q in range(2):  # K pass: chunks (2q, 2q+1)
                    lhsT = ring[
                        :, prev_slab * P + 64 * q : prev_slab * P + 64 * (q + 1)
                    ].rearrange("p (two m) -> p two m", two=2)
                    for j in range(JC):
                        nc.tensor.matmul(
                            G[32 * j : 32 * (j + 1), 0 : hi_c - lo_c],
                            lhsT,
                            w8[q][:, :, j * GF + lo_c : j * GF + hi_c],
                            start=False,
                            stop=(st == n_stage - 1 and q == 1),
                            tile_position=(0, 32 * j),
                            perf_mode=DR,
                            skip_group_check=True,
                        )


def _emit_chain_T(nc, pools, consts, Gs, cur_slab, first_step, emit_o=None):
    """bf16 chain with the cell state kept f-major (c^T [f, 32j+b]).

    tanh(c^T) lands in SBUF already transposed, removing the tct transpose
    from the tail; sf and m are transposed instead.  emit_o(k_lo, k_hi)
    emits the deferred o-block matmuls so the PE order is
    [inputs, fi, g, T_f, o(k 0..2), T_m, o(k=3), T_so] -- neither
    transpose gets stuck behind the full 1.7us stream.
    """
    apool, spool, tpool = pools["a"], pools["s"], pools["t"]
    cT = consts["c"]
    ring_hi = consts["ring_hi"]
    G_fi, G_g, G_o = Gs

    A = apool.tile([P, HGF], F32, tag="A")  # sig(f), sig(i)
    tg = spool.tile([P, P], F32, tag="tg")
    so_t = spool.tile([P, P], BF16, tag="so")
    T2f = tpool.tile([P, P], F32, tag="T2f")
    T2m = tpool.tile([P, P], F32, tag="T2m")
    T2so = tpool.tile([P, P], BF16, tag="T2so")
    tctT = spool.tile([P, P], BF16, tag="tctT")

    # ACT queue: sig(f,i) -> tanh(g) -> sig(o) -> tanh(c^T)
    nc.scalar.activation(A[:, :], G_fi[:, :], AF.Sigmoid)
    nc.scalar.activation(tg, G_g[:, :], AF.Tanh)
    # transpose sf unconditionally (first step ignores it) so the tile pool
    # sees an allocation in every scope
    nc.tensor.transpose(T2f, A[:, 0:P], consts["identF"])
    if emit_o is not None:
        emit_o(0, KC - 1)

    m = spool.tile([P, P], F32, tag="m")
    if first_step:
        nc.vector.tensor_mul(m, A[:, P : 2 * P], tg)
        nc.tensor.transpose(T2m, m, consts["identF"])
        if emit_o is not None:
            emit_o(KC - 1, KC)
        nc.scalar.activation(so_t, G_o[:, :], AF.Sigmoid)
        nc.vector.tensor_copy(cT, T2m)
    else:
        nc.vector.tensor_mul(cT, T2f, cT)
        nc.vector.tensor_mul(m, A[:, P : 2 * P], tg)
        nc.tensor.transpose(T2m, m, consts["identF"])
        if emit_o is not None:
            emit_o(KC - 1, KC)
        nc.scalar.activation(so_t, G_o[:, :], AF.Sigmoid)
        nc.vector.tensor_add(cT, cT, T2m)
    nc.scalar.activation(tctT, cT, AF.Tanh)

    nc.tensor.transpose(T2so, so_t, consts["ident"])
    soT = spool.tile([P, P], BF16, tag="soT")
    nc.vector.tensor_copy(soT, T2so)

    slab_hi = ring_hi[:, cur_slab * P : (cur_slab + 1) * P]
    # chunk pair (0,1) first so the next step's k=0 matmuls can begin
    nc.vector.tensor_mul(slab_hi[:, 0:64], soT[:, 0:64], tctT[:, 0:64])
    nc.vector.tensor_mul(slab_hi[:, 64:P], soT[:, 64:P], tctT[:, 64:P])


def _emit_chain(nc, pools, consts, Gs, cur_slab, first_step, mode):
    """Activations + c update + h^T slab production for one step."""
    apool, spool, tpool = pools["a"], pools["s"], pools["t"]
    c_tile = consts["c"]
    ring_hi = consts["ring_hi"]
    G_fi, G_g, G_o = Gs

    A = apool.tile([P, HGF], F32, tag="A")  # sig(f), sig(i)
    tg = spool.tile([P, P], F32, tag="tg")
    so_t = spool.tile([P, P], BF16, tag="so")
    tct = spool.tile([P, P], BF16, tag="tct")
    T2 = tpool.tile([P, 2 * P], BF16, tag="T2")

    gsc = 1.0 / WSCALE if mode != "bf16" else 1.0
    # ACT queue: sig(f,i) -> tanh(g) -> sig(o) -> tanh(c)
    nc.scalar.activation(A[:, :], G_fi[:, :], AF.Sigmoid, scale=gsc)
    nc.scalar.activation(tg, G_g[:, :], AF.Tanh, scale=gsc)
    nc.scalar.activation(so_t, G_o[:, :], AF.Sigmoid, scale=gsc)

    if first_step:
        # c_prev = 0: c = sig(i) * tanh(g)
        nc.vector.tensor_mul(c_tile, A[:, P : 2 * P], tg)
    else:
        nc.vector.tensor_mul(c_tile, A[:, 0:P], c_tile)
        m = spool.tile([P, P], F32, tag="m")
        nc.vector.tensor_mul(m, A[:, P : 2 * P], tg)
        nc.vector.tensor_add(c_tile, c_tile, m)
    nc.scalar.activation(tct, c_tile, AF.Tanh)

    nc.tensor.transpose(T2[:, 0:P], so_t, consts["ident"])
    soT = spool.tile([P, P], BF16, tag="soT")
    nc.vector.tensor_copy(soT, T2[:, 0:P])
    nc.tensor.transpose(T2[:, P : 2 * P], tct, consts["ident"])
    tctT = T2[:, P : 2 * P]

    slab_hi = ring_hi[:, cur_slab * P : (cur_slab + 1) * P]
    if mode == "fp8split":
        ring_lo = consts["ring_lo"]
        slab_lo = ring_lo[:, cur_slab * P : (cur_slab + 1) * P]
        full = spool.tile([P, P], F32, tag="full")
        dres = spool.tile([P, P], F32, tag="dres")
        nc.vector.tensor_mul(full, soT, tctT)
        # chunk pair (0,1) first so the next step's q=0 matmuls can begin
        nc.vector.tensor_copy(slab_hi[:, 0:64], full[:, 0:64])
        nc.vector.tensor_copy(slab_hi[:, 64:P], full[:, 64:P])
        nc.vector.tensor_sub(dres, full, slab_hi)
        nc.vector.tensor_scalar_mul(slab_lo[:, 0:64], dres[:, 0:64], WSCALE)
        nc.vector.tensor_scalar_mul(slab_lo[:, 64:P], dres[:, 64:P], WSCALE)
    else:
        nc.vector.tensor_mul(slab_hi[:, 0:64], soT[:, 0:64], tctT[:, 0:64])
        nc.vector.tensor_mul(slab_hi[:, 64:P], soT[:, 64:P], tctT[:, 64:P])
        if mode == "fp8":
            ring_ys = consts["ring_ys"]
            slab_ys = ring_ys[:, cur_slab * P : (cur_slab + 1) * P]
            nc.vector.tensor_mul(slab_ys, soT, tctT)


def _ys_window(nc, pools, consts, w, dY, mode, nsteps=WIN):
    """Apply Linear to the h^T slabs of decoder window w and DMA the ys out."""
    ypool, ysb_pool = pools["y"], pools["ysb"]
    sLW, sLB = consts["LW"], consts["LB"]
    if mode == "fp8split":
        rnames = [("ring_hi5", "LW"), ("ring_lo5", "LWs")]
    elif mode == "fp8":
        rnames = [("ring_ys5", "LW")]
    else:
        rnames = [("ring_hi5", "LW")]
    half = w % 2
    yps = ypool.tile([1, WIN * BL], F32, tag="yps")
    n_r = len(rnames)
    for r, (rn, lwn) in enumerate(rnames):
        ring5 = consts[rn]  # [P, 2, WIN, KC, BL]
        lw = consts[lwn]
        for k in range(KC):
            nc.tensor.matmul(
                yps[0:1, 0 : nsteps * BL],
                lw[:, k : k + 1],
                ring5[:, half, 0:nsteps, k, :],
                start=(r == 0 and k == 0),
                stop=(r == n_r - 1 and k == KC - 1),
            )
    ysb = ysb_pool.tile([1, WIN * BL], F32, tag="ysb")
    nc.scalar.activation(
        ysb[0:1, 0 : nsteps * BL], yps[0:1, 0 : nsteps * BL], AF.Identity,
        bias=sLB[0:1, 0:1],
    )
    nc.sync.dma_start(
        dY[0:1, w * WIN * BL : w * WIN * BL + nsteps * BL],
        ysb[0:1, 0 : nsteps * BL],
    )


def build_nc(t_enc=T, t_dec=T, mode="fp8split"):
    nc = bacc.Bacc()
    tmax = max(t_enc, t_dec)
    nblk = (tmax + SPK - 1) // SPK

    dXU = nc.declare_dram_parameter("XU", [P, nblk * P], BF16, isOutput=False)
    dUWE = nc.declare_dram_parameter("UWE", [P, GF], BF16, isOutput=False)
    dUWD = nc.declare_dram_parameter("UWD", [P, GF], BF16, isOutput=False)
    if mode == "bf16":
        dWE = nc.declare_dram_parameter("WE", [KC, P, 4 * GF], BF16, isOutput=False)
        dWD = nc.declare_dram_parameter("WD", [KC, P, 4 * GF], BF16, isOutput=False)
    else:
        dW8E = nc.declare_dram_parameter("W8E", [2, P, 2 * 4 * GF], F8, isOutput=False)
        dW8D = nc.declare_dram_parameter("W8D", [2, P, 2 * 4 * GF], F8, isOutput=False)
        dW8L = {}
        if mode == "fp8split":
            dW8L["E"] = nc.declare_dram_parameter("W8LE", [2, P, 2 * 4 * GF], F8, isOutput=False)
            dW8L["D"] = nc.declare_dram_parameter("W8LD", [2, P, 2 * 4 * GF], F8, isOutput=False)
    dLW = nc.declare_dram_parameter("LW", [P, KC], BF16, isOutput=False)
    dLB = nc.declare_dram_parameter("LB", [1, 1], F32, isOutput=False)
    dY = nc.declare_dram_parameter("Y", [1, t_dec * BL], F32, isOutput=True)

    with ExitStack() as ctx:
        tc = ctx.enter_context(tile.TileContext(nc))
        const = ctx.enter_context(tc.tile_pool(name="const", bufs=1))
        gpool = ctx.enter_context(tc.tile_pool(name="g", bufs=1, space="PSUM"))
        tpool = ctx.enter_context(tc.tile_pool(name="tps", bufs=1, space="PSUM"))
        ypool = ctx.enter_context(tc.tile_pool(name="yps", bufs=1, space="PSUM"))
        apool = ctx.enter_context(tc.tile_pool(name="act", bufs=3))
        spool = ctx.enter_context(tc.tile_pool(name="small", bufs=3))
        ysb_pool = ctx.enter_context(tc.tile_pool(name="ysb", bufs=2))

        consts = {}
        sXU = const.tile([P, nblk * P], BF16, tag="sXU", name="sXU")
        nc.sync.dma_start(sXU[:, :], dXU[:, :])
        consts["XU"] = sXU
        for nm, d in (("UWE", dUWE), ("UWD", dUWD)):
            s = const.tile([P, GF], BF16, tag=f"s{nm}", name=f"s{nm}")
            nc.sync.dma_start(s[:, :], d[:, :])
            consts[nm] = s
        if mode == "bf16":
            for nm, d in (("WE", dWE), ("WD", dWD)):
                tiles = [
                    const.tile([P, 4 * GF], BF16, tag=f"s{nm}{k}", name=f"s{nm}{k}") for k in range(KC)
                ]
                for k in range(KC):
                    nc.sync.dma_start(tiles[k][:, :], d[k])
                consts[nm] = tiles
        else:
            pairs = [("W8E", dW8E), ("W8D", dW8D)]
            if mode == "fp8split":
                pairs += [("W8LE", dW8L["E"]), ("W8LD", dW8L["D"])]
            for nm, d in pairs:
                tiles = []
                for q in range(2):
                    s = const.tile([P, 2 * 4 * GF], F8, tag=f"s{nm}{q}", name=f"s{nm}{q}")
                    nc.sync.dma_start(s[:, :], d[q])
                    tiles.append(s.rearrange("p (two n) -> p two n", two=2))
                consts[nm] = tiles
        sLW = const.tile([P, KC], BF16, tag="sLW")
        nc.sync.dma_start(sLW[:, :], dLW[:, :])
        consts["LW"] = sLW
        if mode == "fp8split":
            sLWs = const.tile([P, KC], BF16, tag="sLWs")
            nc.scalar.activation(sLWs, sLW, AF.Identity, scale=1.0 / WSCALE)
            consts["LWs"] = sLWs
        sLB = const.tile([1, 1], F32, tag="sLB")
        nc.sync.dma_start(sLB[:, :], dLB[:, :])
        consts["LB"] = sLB
        ident = const.tile([P, P], BF16, tag="ident")
        make_identity(nc, ident)
        consts["ident"] = ident
        identF = const.tile([P, P], F32, tag="identF")
        make_identity(nc, identF)
        consts["identF"] = identF
        consts["c"] = const.tile([P, P], F32, tag="c", name="c_tile")

        ring_dt = BF16 if mode == "bf16" else F8
        ring_hi = const.tile([P, RING * P], ring_dt, tag="ring_hi")
        consts["ring_hi"] = ring_hi
        consts["ring_hi5"] = ring_hi.rearrange("p (u s k b) -> p u s k b", u=2, s=WIN, k=KC)
        if mode == "fp8split":
            ring_lo = const.tile([P, RING * P], F8, tag="ring_lo")
            consts["ring_lo"] = ring_lo
            consts["ring_lo5"] = ring_lo.rearrange(
                "p (u s k b) -> p u s k b", u=2, s=WIN, k=KC
            )
        elif mode == "fp8":
            ring_ys = const.tile([P, RING * P], BF16, tag="ring_ys")
            consts["ring_ys"] = ring_ys
            consts["ring_ys5"] = ring_ys.rearrange(
                "p (u s k b) -> p u s k b", u=2, s=WIN, k=KC
            )

        pools = {"a": apool, "s": spool, "t": tpool, "y": ypool, "ysb": ysb_pool}

        n_tot = t_enc + t_dec

        def slab_of(u):
            # decoder slabs restart at 0 so ys windows alternate ring halves
            return u % RING if u < t_enc else (u - t_enc) % RING

        def new_gs():
            return (
                gpool.tile([P, HGF], F32, tag="Gfi", name="Gfi"),
                gpool.tile([P, P], F32, tag="Gg", name="Gg"),
                gpool.tile([P, P], F32, tag="Go", name="Go"),
            )

        # Matmul block for global step 0 (encoder t=0: input only, no recurrent)
        Gs = new_gs()
        _emit_mms(nc, Gs, consts, True, 0, 0, mode, skip_rec=True)
        for u in range(n_tot):
            t = u if u < t_enc else u - t_enc
            first = (u == 0) or (u == t_enc)
            if mode == "bf16":
                eo = None
                if not first:
                    is_enc_u = u < t_enc
                    prev_u = slab_of(u - 1)
                    Gs_u = Gs

                    def eo(k_lo, k_hi, Gs_u=Gs_u, is_enc_u=is_enc_u, prev_u=prev_u):
                        _emit_rec_o(nc, Gs_u, consts, is_enc_u, prev_u, k_lo, k_hi)

                _emit_chain_T(nc, pools, consts, Gs, slab_of(u), first, eo)
            else:
                _emit_chain(nc, pools, consts, Gs, slab_of(u), first, mode)
            if u + 1 < n_tot:
                t_next = u + 1 if u + 1 < t_enc else u + 1 - t_enc
                Gs = new_gs()
                next_first = (u + 1 == t_enc)
                _emit_mms(nc, Gs, consts, u + 1 < t_enc, t_next, slab_of(u), mode,
                          split_o=(mode == "bf16" and not next_first))
            if u >= t_enc:
                td = u - t_enc
                if td % WIN == WIN - 1:
                    _ys_window(nc, pools, consts, td // WIN, dY, mode)
                elif td == t_dec - 1:
                    _ys_window(
                        nc, pools, consts, td // WIN, dY, mode, nsteps=(td % WIN) + 1
                    )

    if not nc.is_finalized():
        nc.finalize()
    return nc


def prep_core_inputs(x_core, weights, mode="fp8split", t=T):
    """Host-side layout prep for one core. x_core: [BL, t, 1] fp32."""
    import ml_dtypes

    bf = ml_dtypes.bfloat16
    f8 = mybir.dt.np(F8)
    perm = _perm_fold()
    out = {}

    nblk = (t + SPK - 1) // SPK
    xu = np.zeros((P, nblk * P), dtype=np.float32)
    xs = x_core[:, :t, 0]  # [BL, t]
    for tt in range(t):
        blk, s = tt // SPK, tt % SPK
        for j in range(JC):
            xu[8 * s + 2 * j, blk * P + 32 * j : blk * P + 32 * j + 32] = xs[:, tt]
            xu[8 * s + 2 * j + 1, blk * P + 32 * j : blk * P + 32 * j + 32] = 1.0
    out["XU"] = xu.astype(bf)

    for tag, Wih, Whh, bih, bhh in (
        ("E", weights["enc_Wih"], weights["enc_Whh"], weights["enc_bih"], weights["enc_bhh"]),
        ("D", weights["dec_Wih"], weights["dec_Whh"], weights["dec_bih"], weights["dec_bhh"]),
    ):
        wsc = WSCALE if mode != "bf16" else 1.0
        uw = np.zeros((P, GF), dtype=np.float32)
        wf = np.asarray(Wih[perm, 0], dtype=np.float32) * wsc  # [4H] folded
        bf_ = np.asarray((bih + bhh)[perm], dtype=np.float32) * wsc
        for j in range(JC):
            uw[2 * j] = wf[j * GF : (j + 1) * GF]
            uw[2 * j + 1] = bf_[j * GF : (j + 1) * GF]
        out["UW" + tag] = uw.astype(bf)

        Wf = np.ascontiguousarray(Whh[perm, :].T)  # [H, 4H] folded cols
        if mode == "bf16":
            out["W" + tag] = Wf.reshape(KC, P, 4 * GF).astype(bf)
        else:
            # [2 pass, 128 p, 2 i, 2048 n]: W8[q][p, i, n] = Wf[256q+128i+p, n]
            w8 = Wf.reshape(2, 2, P, 4 * GF).transpose(0, 2, 1, 3).reshape(2, P, 2 * 4 * GF)
            out["W8" + tag] = np.ascontiguousarray(w8 * WSCALE).astype(f8)
            if mode == "fp8split":
                out["W8L" + tag] = np.ascontiguousarray(w8).astype(f8)
    out["LW"] = np.ascontiguousarray(weights["lin_W"][0].reshape(KC, P).T).astype(bf)
    out["LB"] = weights["lin_b"].reshape(1, 1).astype(np.float32)
    return out


_CACHE = {}
_LAST_RESULTS = None
MODE = "bf16"


def kernel(**inputs) -> np.ndarray:
    global _LAST_RESULTS
    mode = MODE
    key = ("full", mode)
    if key not in _CACHE:
        _CACHE[key] = build_nc(T, T, mode)
    nc = _CACHE[key]

    x = np.asarray(inputs["x"], dtype=np.float32)
    in_maps = [
        prep_core_inputs(x[i * BL : (i + 1) * BL], inputs, mode)
        for i in range(N_CORES)
    ]

    res = run_bass_kernel_spmd(nc, in_maps, core_ids=list(range(N_CORES)))
    _LAST_RESULTS = res
    y = np.empty((B, T, 1), dtype=np.float32)
    for i in range(N_CORES):
        yi = np.asarray(res.results[i]["Y"], dtype=np.float32).reshape(T, BL)
        y[i * BL : (i + 1) * BL, :, 0] = yi.T
    return y
